# revision 3
# baseline (speedup 1.0000x reference)
"""Trainium2 Bass kernel for nn_ClassificationHead: LayerNorm -> Linear(1024,256) -> GELU -> Linear(256,2).

Data-parallel over 8 NeuronCores: each core processes 8192 rows of the
65536-row batch; the tiny weights are replicated.

Per-core pipeline (per 128-row tile, grouped by 8 for the stats math):
  1. SWDGE cast-DMA loads the fp32 rows from HBM as bf16 into a natural-layout
     SBUF tile [128 rows, 1024+128 cols].
  2. DVE bn_stats/bn_aggr computes per-row mean/var; per 8-tile group a
     batched Newton-rsqrt (bit-trick seed + 2 iterations, all on DVE) gives
     g = 1/sqrt(var+eps) and rhat = 1/g; (-mu, rhat) land in two spare
     columns of the natural tile.
  3. One HWDGE xbar-transpose DMA produces the K-major tile [128, 9, 128];
     the stats columns become two extra contraction rows.
  4. TensorE: 8 accumulating matmuls (x @ W1') + a rank-2 matmul with
     rhs=[s1; c1] that adds (-mu*s1 + rhat*c1) — so after scaling by g
     (fused into the GELU's per-partition scale) the PSUM holds exactly
     LN(x) @ W1' + b1'.
  5. ACT evaluates exact GELU with per-partition scale g -> bf16 h tile.
  6. TensorE transposes h (via identity), ACT evacuates PSUM->SBUF bf16,
     TensorE computes h @ W2; DVE adds b2 into a staging tile.
  7. One DMA writes the [8192, 2] fp32 result back.

Weight folding done on host (tiny, O(1MB)): W1' = ln_w[:,None]*W1,
s1 = colsum(W1'), c1 = ln_b@W1 + b1.
"""
import sys

sys.path.insert(0, "/opt/trn_rl_repo")
sys.path.insert(0, "/root/.axon_site")

import numpy as np
import ml_dtypes

N_CORES = 8
BATCH = 65536
D = 1024
H = 256
OUT = 2
RPC = BATCH // N_CORES  # rows per core
NT = RPC // 128         # 128-row tiles per core
KC = D // 128           # contraction chunks
G = 8                   # tiles per stats group
EPS = 1e-5
MAGIC = 0x5F3759DF

_cache = {}


def _bf16(a):
    return np.asarray(a, dtype=ml_dtypes.bfloat16)


def _build():
    import concourse.bacc as bacc
    import concourse.mybir as mybir
    from concourse import tile

    f32 = mybir.dt.float32
    i32 = mybir.dt.int32
    bf16 = mybir.dt.bfloat16
    AF = mybir.ActivationFunctionType
    ALU = mybir.AluOpType

    nc = bacc.Bacc(None, target_bir_lowering=False, debug=False)

    x_in = nc.dram_tensor("x", [RPC, D], f32, kind="ExternalInput")
    w1_in = nc.dram_tensor("w1b", [128, KC, H], bf16, kind="ExternalInput")
    sc_in = nc.dram_tensor("sc", [2, H], bf16, kind="ExternalInput")
    w2_in = nc.dram_tensor("w2b", [128, 2, OUT], bf16, kind="ExternalInput")
    b2_in = nc.dram_tensor("b2r", [128, OUT], f32, kind="ExternalInput")
    id_in = nc.dram_tensor("ident", [128, 128], bf16, kind="ExternalInput")
    y_out = nc.dram_tensor("y", [RPC, OUT], f32, kind="ExternalOutput")

    x_t = x_in.rearrange("(t p) d -> t p d", p=128)
    y_t = y_out.rearrange("(t p) c -> p t c", p=128)

    with tile.TileContext(nc) as tc:
        with (
            tc.tile_pool(name="wpool", bufs=1) as wp,
            tc.tile_pool(name="natp", bufs=11) as natp,
            tc.tile_pool(name="xtp", bufs=4) as xtp,
            tc.tile_pool(name="statp", bufs=2) as statp,
            tc.tile_pool(name="hbp", bufs=3) as hbp,
            tc.tile_pool(name="htp", bufs=3) as htp,
            tc.tile_pool(name="outp", bufs=1) as outp,
            tc.tile_pool(name="pszp", bufs=4, space="PSUM") as pszp,
            tc.tile_pool(name="pstp", bufs=2, space="PSUM") as pstp,
            tc.tile_pool(name="psop", bufs=2, space="PSUM") as psop,
        ):
            w1sb = wp.tile([128, KC, H], bf16)
            nc.sync.dma_start(w1sb[:], w1_in[:])
            scsb = wp.tile([2, H], bf16)
            nc.sync.dma_start(scsb[:], sc_in[:])
            w2sb = wp.tile([128, 2, OUT], bf16)
            nc.sync.dma_start(w2sb[:], w2_in[:])
            b2sb = wp.tile([128, OUT], f32)
            nc.sync.dma_start(b2sb[:], b2_in[:])
            idsb = wp.tile([128, 128], bf16)
            nc.sync.dma_start(idsb[:], id_in[:])

            outsb = outp.tile([128, NT, OUT], f32)

            for grp in range(NT // G):
                nats = []
                S = statp.tile([128, G, 2], f32, tag="S")
                for t8 in range(G):
                    t = grp * G + t8
                    natb = natp.tile([128, D + 128], bf16, tag="natb")
                    nats.append(natb)
                    nc.gpsimd.dma_start(natb[:, 0:D], x_t[t])
                    bst = statp.tile([128, 12], f32, tag="bst")
                    nc.vector.bn_stats(bst[:, 0:6], natb[:, 0:512])
                    nc.vector.bn_stats(bst[:, 6:12], natb[:, 512:1024])
                    nc.vector.bn_aggr(S[:, t8, :], bst[:])

                # Batched per-group stats math (all DVE, [128, G] tiles):
                # g = rsqrt(var+eps) via bit-trick seed + 2 Newton steps.
                V = statp.tile([128, G], f32, tag="V")
                nc.vector.tensor_scalar_add(V[:], S[:, :, 1], EPS)
                Y = statp.tile([128, G], f32, tag="Y")
                Yi = Y[:].bitcast(i32)
                T = statp.tile([128, G], f32, tag="T")
                Ti = T[:].bitcast(i32)
                nc.vector.tensor_scalar(Ti, V[:].bitcast(i32), 1, None, ALU.logical_shift_right)
                nc.vector.tensor_scalar(Yi, Ti, -1, MAGIC, ALU.mult, ALU.add)
                for _ in range(2):
                    nc.vector.tensor_tensor(T[:], V[:], Y[:], ALU.mult)
                    nc.vector.tensor_tensor(T[:], T[:], Y[:], ALU.mult)
                    nc.vector.tensor_scalar(T[:], T[:], -0.5, 1.5, ALU.mult, ALU.add)
                    nc.vector.tensor_tensor(Y[:], Y[:], T[:], ALU.mult)
                # BM[:, :, 0] = -mu (bf16), BM[:, :, 1] = rhat = (var+eps)*g (bf16)
                BM = statp.tile([128, G, 2], bf16, tag="BM")
                nc.vector.tensor_scalar_mul(BM[:, :, 0], S[:, :, 0], -1.0)
                nc.vector.tensor_tensor(BM[:, :, 1], V[:], Y[:], ALU.mult)

                for t8 in range(G):
                    t = grp * G + t8
                    natb = nats[t8]
                    nc.vector.tensor_copy(natb[:, D : D + 2], BM[:, t8, :])

                    xt = xtp.tile([128, KC + 1, 128], bf16, tag="xt")
                    nc.sync.dma_start(xt[:], natb[:], transpose=True)

                    psz = pszp.tile([128, H], f32, tag="psz")
                    for k in range(KC):
                        nc.tensor.matmul(
                            psz[:], xt[:, k, :], w1sb[:, k, :], start=(k == 0), stop=False
                        )
                    nc.tensor.matmul(psz[:], xt[0:2, KC, :], scsb[:], start=False, stop=True)

                    hb = hbp.tile([128, H], bf16, tag="hb")
                    nc.scalar.activation(
                        hb[:], psz[:], AF.Gelu, bias=0.0, scale=Y[:, t8 : t8 + 1]
                    )

                    pst = pstp.tile([128, H], bf16, tag="pst")
                    nc.tensor.transpose(pst[:, 0:128], hb[:, 0:128], idsb[:])
                    nc.tensor.transpose(pst[:, 128:256], hb[:, 128:256], idsb[:])
                    ht = htp.tile([128, 2, 128], bf16, tag="ht")
                    nc.scalar.copy(ht[:], pst[:])

                    pso = psop.tile([128, OUT], f32, tag="pso")
                    nc.tensor.matmul(pso[:], ht[:, 0, :], w2sb[:, 0, :], start=True, stop=False)
                    nc.tensor.matmul(pso[:], ht[:, 1, :], w2sb[:, 1, :], start=False, stop=True)

                    nc.vector.tensor_add(outsb[:, t, :], pso[:], b2sb[:])

            nc.sync.dma_start(y_t[:], outsb[:])

    nc.finalize()
    return nc


def _get_nc():
    if "nc" not in _cache:
        _cache["nc"] = _build()
    return _cache["nc"]


def _prep_weights(ln_w, ln_b, W1, b1, W2, b2):
    W1p = ln_w[:, None] * W1                      # [1024, 256]
    s1 = W1p.sum(axis=0)                          # [256]
    c1 = ln_b @ W1 + b1                           # [256]
    return {
        "w1b": _bf16(W1p.reshape(KC, 128, H).transpose(1, 0, 2)),
        "sc": _bf16(np.stack([s1, c1])),
        "w2b": _bf16(W2.reshape(2, 128, OUT).transpose(1, 0, 2)),
        "b2r": np.broadcast_to(b2, (128, OUT)).astype(np.float32).copy(),
        "ident": _bf16(np.eye(128, dtype=np.float32)),
    }


def kernel(embedding, ln_w, ln_b, W1, b1, W2, b2):
    from concourse.bass_utils import run_bass_kernel_spmd

    embedding = np.asarray(embedding, dtype=np.float32)
    weights = _prep_weights(
        np.asarray(ln_w, dtype=np.float32), np.asarray(ln_b, dtype=np.float32),
        np.asarray(W1, dtype=np.float32), np.asarray(b1, dtype=np.float32),
        np.asarray(W2, dtype=np.float32), np.asarray(b2, dtype=np.float32),
    )
    nc = _get_nc()
    shards = embedding.reshape(N_CORES, RPC, D)
    in_maps = [{"x": shards[c], **weights} for c in range(N_CORES)]
    res = run_bass_kernel_spmd(nc, in_maps, core_ids=list(range(N_CORES)))
    out = np.concatenate([res.results[c]["y"] for c in range(N_CORES)], axis=0)
    return out.astype(np.float32)


# revision 4
# speedup vs baseline: 1.3779x; 1.3779x over previous
"""Trainium2 Bass kernel for nn_ClassificationHead: LayerNorm -> Linear(1024,256) -> GELU -> Linear(256,2).

Data-parallel over 8 NeuronCores: each core processes 8192 rows of the
65536-row batch; the tiny weights are replicated.

Per-core pipeline (per 128-row tile; tiles processed in quads so each
xbar-transpose DMA reads >=4KB per partition line, and stats math is
batched per 8-tile group):
  1. One SWDGE cast-DMA loads 4 tiles (512 rows) of fp32 from HBM as bf16
     into a quad natural-layout SBUF tile [128, 4*(1024+128)].
  2. DVE bn_stats/bn_aggr computes per-row mean/var; per 8-tile group a
     batched Newton-rsqrt (bit-trick seed + 2 iterations, all on DVE) gives
     g = 1/sqrt(var+eps) and rhat = 1/g; (-mu, rhat) land in two spare
     columns of each sub-tile.
  3. One HWDGE xbar-transpose DMA per quad produces K-major tiles
     [128, 36, 128]; the stats columns become extra contraction rows.
  4. TensorE per sub-tile: 8 accumulating matmuls (x @ W1') + a rank-2
     matmul with rhs=[s1; c1] adding (-mu*s1 + rhat*c1) — after scaling by
     g (fused into the GELU's per-partition scale) the PSUM holds exactly
     LN(x) @ W1' + b1.
  5. ACT evaluates exact GELU with per-partition scale g -> bf16 h tile.
  6. TensorE transposes h (via identity), ACT evacuates PSUM->SBUF bf16,
     TensorE computes h @ W2; DVE adds b2 into a staging tile.
  7. One DMA writes the [8192, 2] fp32 result back.

Weight folding done on host (tiny, O(1MB)): W1' = ln_w[:,None]*W1,
s1 = colsum(W1'), c1 = ln_b@W1 + b1.
"""
import sys

sys.path.insert(0, "/opt/trn_rl_repo")
sys.path.insert(0, "/root/.axon_site")

import numpy as np
import ml_dtypes

N_CORES = 8
BATCH = 65536
D = 1024
W = D + 128             # sub-tile width incl. stats columns
H = 256
OUT = 2
RPC = BATCH // N_CORES  # rows per core
NT = RPC // 128         # 128-row tiles per core
KC = D // 128           # contraction chunks
Q = 4                   # tiles per transpose quad
G = 8                   # tiles per stats group (2 quads)
EPS = 1e-5
MAGIC = 0x5F3759DF

_cache = {}


def _bf16(a):
    return np.asarray(a, dtype=ml_dtypes.bfloat16)


def _build():
    import concourse.bacc as bacc
    import concourse.mybir as mybir
    from concourse import tile

    f32 = mybir.dt.float32
    i32 = mybir.dt.int32
    bf16 = mybir.dt.bfloat16
    AF = mybir.ActivationFunctionType
    ALU = mybir.AluOpType

    nc = bacc.Bacc(None, target_bir_lowering=False, debug=False)

    x_in = nc.dram_tensor("x", [RPC, D], f32, kind="ExternalInput")
    w1_in = nc.dram_tensor("w1b", [128, KC, H], bf16, kind="ExternalInput")
    sc_in = nc.dram_tensor("sc", [2, H], bf16, kind="ExternalInput")
    w2_in = nc.dram_tensor("w2b", [128, 2, OUT], bf16, kind="ExternalInput")
    b2_in = nc.dram_tensor("b2r", [128, OUT], f32, kind="ExternalInput")
    id_in = nc.dram_tensor("ident", [128, 128], bf16, kind="ExternalInput")
    y_out = nc.dram_tensor("y", [RPC, OUT], f32, kind="ExternalOutput")

    x_q = x_in.rearrange("(u q p) d -> u p q d", p=128, q=Q)
    y_t = y_out.rearrange("(t p) c -> p t c", p=128)

    with tile.TileContext(nc) as tc:
        with (
            tc.tile_pool(name="wpool", bufs=1) as wp,
            tc.tile_pool(name="natp", bufs=4) as natp,
            tc.tile_pool(name="xtp", bufs=3) as xtp,
            tc.tile_pool(name="statp", bufs=2) as statp,
            tc.tile_pool(name="hbp", bufs=3) as hbp,
            tc.tile_pool(name="htp", bufs=3) as htp,
            tc.tile_pool(name="outp", bufs=1) as outp,
            tc.tile_pool(name="pszp", bufs=4, space="PSUM") as pszp,
            tc.tile_pool(name="pstp", bufs=2, space="PSUM") as pstp,
            tc.tile_pool(name="psop", bufs=2, space="PSUM") as psop,
        ):
            w1sb = wp.tile([128, KC, H], bf16)
            nc.sync.dma_start(w1sb[:], w1_in[:])
            scsb = wp.tile([2, H], bf16)
            nc.sync.dma_start(scsb[:], sc_in[:])
            w2sb = wp.tile([128, 2, OUT], bf16)
            nc.sync.dma_start(w2sb[:], w2_in[:])
            b2sb = wp.tile([128, OUT], f32)
            nc.sync.dma_start(b2sb[:], b2_in[:])
            idsb = wp.tile([128, 128], bf16)
            nc.sync.dma_start(idsb[:], id_in[:])

            outsb = outp.tile([128, NT, OUT], f32)

            for grp in range(NT // G):
                nats = {}
                S = statp.tile([128, G, 2], f32, tag="S")
                for u2 in range(G // Q):
                    u = grp * (G // Q) + u2
                    natb = natp.tile([128, Q, W], bf16, tag="natb")
                    nats[u2] = natb
                    nc.gpsimd.dma_start(natb[:, :, 0:D], x_q[u])
                    for q in range(Q):
                        t8 = u2 * Q + q
                        bst = statp.tile([128, 12], f32, tag="bst")
                        nc.vector.bn_stats(bst[:, 0:6], natb[:, q, 0:512])
                        nc.vector.bn_stats(bst[:, 6:12], natb[:, q, 512:1024])
                        nc.vector.bn_aggr(S[:, t8, :], bst[:])

                # Batched per-group stats math (all DVE, [128, G] tiles):
                # g = rsqrt(var+eps) via bit-trick seed + 2 Newton steps.
                V = statp.tile([128, G], f32, tag="V")
                nc.vector.tensor_scalar_add(V[:], S[:, :, 1], EPS)
                Y = statp.tile([128, G], f32, tag="Y")
                Yi = Y[:].bitcast(i32)
                T = statp.tile([128, G], f32, tag="T")
                Ti = T[:].bitcast(i32)
                nc.vector.tensor_scalar(Ti, V[:].bitcast(i32), 1, None, ALU.logical_shift_right)
                nc.vector.tensor_scalar(Yi, Ti, -1, MAGIC, ALU.mult, ALU.add)
                for _ in range(2):
                    nc.vector.tensor_tensor(T[:], V[:], Y[:], ALU.mult)
                    nc.vector.tensor_tensor(T[:], T[:], Y[:], ALU.mult)
                    nc.vector.tensor_scalar(T[:], T[:], -0.5, 1.5, ALU.mult, ALU.add)
                    nc.vector.tensor_tensor(Y[:], Y[:], T[:], ALU.mult)
                # BM[:, :, 0] = -mu (bf16), BM[:, :, 1] = rhat = (var+eps)*g (bf16)
                BM = statp.tile([128, G, 2], bf16, tag="BM")
                nc.vector.tensor_scalar_mul(BM[:, :, 0], S[:, :, 0], -1.0)
                nc.vector.tensor_tensor(BM[:, :, 1], V[:], Y[:], ALU.mult)

                for u2 in range(G // Q):
                    natb = nats[u2]
                    for q in range(Q):
                        t8 = u2 * Q + q
                        nc.vector.tensor_copy(natb[:, q, D : D + 2], BM[:, t8, :])

                    xt = xtp.tile([128, Q * (KC + 1), 128], bf16, tag="xt")
                    nc.sync.dma_start(xt[:], natb[:], transpose=True)

                    for q in range(Q):
                        t8 = u2 * Q + q
                        t = grp * G + t8
                        kb = q * (KC + 1)

                        psz = pszp.tile([128, H], f32, tag="psz")
                        for k in range(KC):
                            nc.tensor.matmul(
                                psz[:], xt[:, kb + k, :], w1sb[:, k, :],
                                start=(k == 0), stop=False,
                            )
                        nc.tensor.matmul(
                            psz[:], xt[0:2, kb + KC, :], scsb[:], start=False, stop=True
                        )

                        hb = hbp.tile([128, H], bf16, tag="hb")
                        nc.scalar.activation(
                            hb[:], psz[:], AF.Gelu, bias=0.0, scale=Y[:, t8 : t8 + 1]
                        )

                        pst = pstp.tile([128, H], bf16, tag="pst")
                        nc.tensor.transpose(pst[:, 0:128], hb[:, 0:128], idsb[:])
                        nc.tensor.transpose(pst[:, 128:256], hb[:, 128:256], idsb[:])
                        ht = htp.tile([128, 2, 128], bf16, tag="ht")
                        nc.scalar.copy(ht[:], pst[:])

                        pso = psop.tile([128, OUT], f32, tag="pso")
                        nc.tensor.matmul(pso[:], ht[:, 0, :], w2sb[:, 0, :], start=True, stop=False)
                        nc.tensor.matmul(pso[:], ht[:, 1, :], w2sb[:, 1, :], start=False, stop=True)

                        nc.vector.tensor_add(outsb[:, t, :], pso[:], b2sb[:])

            nc.sync.dma_start(y_t[:], outsb[:])

    nc.finalize()
    return nc


def _get_nc():
    if "nc" not in _cache:
        _cache["nc"] = _build()
    return _cache["nc"]


def _prep_weights(ln_w, ln_b, W1, b1, W2, b2):
    W1p = ln_w[:, None] * W1                      # [1024, 256]
    s1 = W1p.sum(axis=0)                          # [256]
    c1 = ln_b @ W1 + b1                           # [256]
    return {
        "w1b": _bf16(W1p.reshape(KC, 128, H).transpose(1, 0, 2)),
        "sc": _bf16(np.stack([s1, c1])),
        "w2b": _bf16(W2.reshape(2, 128, OUT).transpose(1, 0, 2)),
        "b2r": np.broadcast_to(b2, (128, OUT)).astype(np.float32).copy(),
        "ident": _bf16(np.eye(128, dtype=np.float32)),
    }


def kernel(embedding, ln_w, ln_b, W1, b1, W2, b2):
    from concourse.bass_utils import run_bass_kernel_spmd

    embedding = np.asarray(embedding, dtype=np.float32)
    weights = _prep_weights(
        np.asarray(ln_w, dtype=np.float32), np.asarray(ln_b, dtype=np.float32),
        np.asarray(W1, dtype=np.float32), np.asarray(b1, dtype=np.float32),
        np.asarray(W2, dtype=np.float32), np.asarray(b2, dtype=np.float32),
    )
    nc = _get_nc()
    shards = embedding.reshape(N_CORES, RPC, D)
    in_maps = [{"x": shards[c], **weights} for c in range(N_CORES)]
    res = run_bass_kernel_spmd(nc, in_maps, core_ids=list(range(N_CORES)))
    out = np.concatenate([res.results[c]["y"] for c in range(N_CORES)], axis=0)
    return out.astype(np.float32)


# revision 17
# speedup vs baseline: 1.4653x; 1.0634x over previous
"""Trainium2 Bass kernel for nn_ClassificationHead: LayerNorm -> Linear(1024,256) -> GELU -> Linear(256,2).

Data-parallel over 8 NeuronCores: each core processes 8192 rows of the
65536-row batch; the tiny weights are replicated. The host supplies each
core's shard pre-transposed in bf16 (layout-only prep: [1024, 8192],
K-major as the tensor engine requires); all math runs on device.

Per-core pipeline, per 512-row block (4 tiles of 128 rows):
  1. One DMA loads the K-major block [128, 8, 512] bf16.
  2. Per tile, TensorE runs 8 accumulating matmuls against W1aug
     ([W1' | ones] -> PSUM cols 0:256 = x @ W1', col 256 = rowsum), plus a
     Gram matmul reusing the already-loaded stationary x-chunk
     (ldweights=False) into PSUM cols 257:385.
  3. DVE extracts -mu (from the rowsum col) and sum(x^2) (Gram diagonal via
     tensor_tensor_reduce against an identity); a batched Newton-rsqrt
     (bit-trick seed + 2 iterations) gives g = 1/sqrt(var+eps) and
     rhat = 1/g. A tiny [128,128] xbar-transpose DMA flips the per-row
     stats into rows.
  4. TensorE adds the rank-2 correction (-mu ox s1 + rhat ox c1), so after
     the GELU's per-partition scale g the PSUM holds exactly LN(x)@W1'+b1.
  5. ACT evaluates exact GELU with scale g -> bf16 h tile.
  6. TensorE transposes h (via identity), ACT evacuates PSUM->SBUF bf16,
     TensorE computes h @ W2; DVE adds b2 into a staging tile.
  7. One DMA writes the [8192, 2] fp32 result back.

Host-side weight folding (tiny, O(1MB)): W1' = ln_w[:,None]*W1,
s1 = colsum(W1'), c1 = ln_b@W1 + b1.
"""
import sys

sys.path.insert(0, "/opt/trn_rl_repo")
sys.path.insert(0, "/root/.axon_site")

import numpy as np
import ml_dtypes

N_CORES = 8
BATCH = 65536
D = 1024
H = 256
OUT = 2
RPC = BATCH // N_CORES  # rows per core
NT = RPC // 128         # 128-row tiles per core
KC = D // 128           # contraction chunks
G = 4                   # tiles per block (512 rows)
NB = NT // G            # blocks per core
EPS = 1e-5
MAGIC = 0x5F3759DF

_cache = {}


def _bf16(a):
    return np.asarray(a, dtype=ml_dtypes.bfloat16)


def _build(rpc=RPC):
    import concourse.bacc as bacc
    import concourse.mybir as mybir
    from concourse import tile

    f32 = mybir.dt.float32
    i32 = mybir.dt.int32
    bf16 = mybir.dt.bfloat16
    AF = mybir.ActivationFunctionType
    ALU = mybir.AluOpType

    nc = bacc.Bacc(None, target_bir_lowering=False, debug=False)

    xt_in = nc.dram_tensor("xt", [D, rpc], bf16, kind="ExternalInput")
    w1_in = nc.dram_tensor("w1aug", [128, KC, H + 1], bf16, kind="ExternalInput")
    sc_in = nc.dram_tensor("screp", [2 * G, G, H + 1], bf16, kind="ExternalInput")
    w2_in = nc.dram_tensor("w2b", [128, 2, OUT], bf16, kind="ExternalInput")
    b2_in = nc.dram_tensor("b2r", [128, OUT], f32, kind="ExternalInput")
    idb_in = nc.dram_tensor("identb", [128, 128], bf16, kind="ExternalInput")
    idf_in = nc.dram_tensor("identf", [128, 128], f32, kind="ExternalInput")
    y_out = nc.dram_tensor("y", [rpc, OUT], f32, kind="ExternalOutput")

    xt_v = xt_in.rearrange("(c p) r -> p c r", p=128)   # [128, KC, RPC]
    y_t = y_out.rearrange("(t p) c -> p t c", p=128)

    with tile.TileContext(nc) as tc:
        with (
            tc.tile_pool(name="wpool", bufs=1) as wp,
            tc.tile_pool(name="xtp", bufs=3) as xtp,
            tc.tile_pool(name="statp", bufs=2) as statp,
            tc.tile_pool(name="scrp", bufs=2) as scrp,
            tc.tile_pool(name="hbp", bufs=3) as hbp,
            tc.tile_pool(name="htp", bufs=3) as htp,
            tc.tile_pool(name="outp", bufs=1) as outp,
            tc.tile_pool(name="pszp", bufs=4, space="PSUM") as pszp,
            tc.tile_pool(name="psgp", bufs=2, space="PSUM") as psgp,
            tc.tile_pool(name="pstp", bufs=1, space="PSUM") as pstp,
            tc.tile_pool(name="psop", bufs=1, space="PSUM") as psop,
        ):
            w1sb = wp.tile([128, KC, H + 1], bf16)
            nc.sync.dma_start(w1sb[:], w1_in[:])
            scsb = wp.tile([2 * G, G, H + 1], bf16)
            nc.sync.dma_start(scsb[:], sc_in[:])
            w2sb = wp.tile([128, 2, OUT], bf16)
            nc.sync.dma_start(w2sb[:], w2_in[:])
            b2sb = wp.tile([128, OUT], f32)
            nc.sync.dma_start(b2sb[:], b2_in[:])
            idbsb = wp.tile([128, 128], bf16)
            nc.sync.dma_start(idbsb[:], idb_in[:])
            idfsb = wp.tile([128, 128], f32)
            nc.sync.dma_start(idfsb[:], idf_in[:])

            nt = rpc // 128
            outsb = outp.tile([128, nt, OUT], f32)

            for u in range(nt // G):
                xtg = xtp.tile([128, KC, G * 128], bf16, tag="xtg")
                nc.sync.dma_start(xtg[:], xt_v[:, :, u * G * 128 : (u + 1) * G * 128])

                S = statp.tile([128, G, 2], f32, tag="S")
                pszs = []
                for q in range(G):
                    rs = q * 128
                    pszg = pszp.tile([128, H + 1], f32, tag="pszg")
                    pszs.append(pszg)
                    psg = psgp.tile([128, 128], f32, tag="psg")
                    for k in range(KC):
                        mm1 = nc.tensor.matmul(
                            pszg[:, 0 : H + 1], xtg[:, k, rs : rs + 128], w1sb[:, k, :],
                            start=(k == 0), stop=False,
                        )
                        nc.tensor.matmul(
                            psg[:],
                            xtg[:, k, rs : rs + 128], xtg[:, k, rs : rs + 128],
                            start=(k == 0), stop=(k == KC - 1),
                        )
                    # -mu and sum(x^2) into the per-block stats tile
                    nc.vector.tensor_scalar_mul(S[:, q, 0:1], pszg[:, H : H + 1], -1.0 / D)
                    scr = scrp.tile([128, 128], f32, tag="scr")
                    nc.vector.scalar_tensor_tensor(
                        scr[:], idfsb[:], 1.0, psg[:],
                        ALU.mult, ALU.mult, accum_out=S[:, q, 1:2],
                    )

                # Batched stats: V = var+eps = SS/D - mu^2 + eps; Y = rsqrt(V).
                A1 = statp.tile([128, G], f32, tag="A1")
                nc.vector.tensor_scalar(A1[:], S[:, :, 1], 1.0 / D, EPS, ALU.mult, ALU.add)
                B = statp.tile([128, G], f32, tag="B")
                nc.vector.tensor_tensor(B[:], S[:, :, 0], S[:, :, 0], ALU.mult)
                V = statp.tile([128, G], f32, tag="V")
                nc.vector.tensor_tensor(V[:], A1[:], B[:], ALU.subtract)
                Y = statp.tile([128, G], f32, tag="Y")
                T = statp.tile([128, G], f32, tag="T")
                nc.vector.tensor_scalar(T[:].bitcast(i32), V[:].bitcast(i32), 1, None, ALU.logical_shift_right)
                nc.vector.tensor_scalar(Y[:].bitcast(i32), T[:].bitcast(i32), -1, MAGIC, ALU.mult, ALU.add)
                for _ in range(2):
                    nc.vector.tensor_tensor(T[:], V[:], Y[:], ALU.mult)
                    nc.vector.tensor_tensor(T[:], T[:], Y[:], ALU.mult)
                    nc.vector.tensor_scalar(T[:], T[:], -0.5, 1.5, ALU.mult, ALU.add)
                    nc.vector.tensor_tensor(Y[:], Y[:], T[:], ALU.mult)

                # BM cols 0:2G = interleaved (-mu, rhat) in bf16; xbar-flip to rows.
                BM = scrp.tile([128, 128], bf16, tag="BM")
                BMv = BM[:, 0 : 2 * G].rearrange("p (q s) -> p q s", s=2)
                nc.vector.tensor_copy(BMv[:, :, 0], S[:, :, 0])
                nc.vector.tensor_tensor(BMv[:, :, 1], V[:], Y[:], ALU.mult)
                BMT = scrp.tile([128, 128], bf16, tag="BMT")
                nc.sync.dma_start(BMT[:], BM[:], transpose=True)

                for q in range(G):
                    t = u * G + q
                    pszg = pszs[q]
                    nc.tensor.matmul(
                        pszg[:, 0 : H + 1], BMT[0 : 2 * G, :],
                        scsb[:, q, :], start=False, stop=True,
                    )
                    hb = hbp.tile([128, H], bf16, tag="hb")
                    nc.scalar.activation(
                        hb[:], pszg[:, 0:H], AF.Gelu, bias=0.0, scale=Y[:, q : q + 1]
                    )

                    pst = pstp.tile([128, H], bf16, tag="pst")
                    nc.tensor.transpose(pst[:, 0:128], hb[:, 0:128], idbsb[:])
                    nc.tensor.transpose(pst[:, 128:256], hb[:, 128:256], idbsb[:])
                    ht = htp.tile([128, 2, 128], bf16, tag="ht")
                    nc.scalar.copy(ht[:], pst[:])

                    pso = psop.tile([128, OUT], f32, tag="pso")
                    nc.tensor.matmul(pso[:], ht[:, 0, :], w2sb[:, 0, :], start=True, stop=False)
                    nc.tensor.matmul(pso[:], ht[:, 1, :], w2sb[:, 1, :], start=False, stop=True)

                    nc.vector.tensor_add(outsb[:, t, :], pso[:], b2sb[:])

            nc.sync.dma_start(y_t[:], outsb[:])

    nc.finalize()
    return nc


def _get_nc():
    if "nc" not in _cache:
        _cache["nc"] = _build()
    return _cache["nc"]


def _prep_weights(ln_w, ln_b, W1, b1, W2, b2):
    W1p = ln_w[:, None] * W1                      # [1024, 256]
    s1 = W1p.sum(axis=0)                          # [256]
    c1 = ln_b @ W1 + b1                           # [256]
    w1aug = np.concatenate([W1p, np.ones((D, 1), np.float32)], axis=1)  # ones col -> rowsum
    sc = np.zeros((2 * G, G, H + 1), np.float32)
    for q in range(G):
        sc[2 * q, q, 0:H] = s1
        sc[2 * q + 1, q, 0:H] = c1
    return {
        "w1aug": _bf16(w1aug.reshape(KC, 128, H + 1).transpose(1, 0, 2)),
        "screp": _bf16(sc),
        "w2b": _bf16(W2.reshape(2, 128, OUT).transpose(1, 0, 2)),
        "b2r": np.broadcast_to(b2, (128, OUT)).astype(np.float32).copy(),
        "identb": _bf16(np.eye(128, dtype=np.float32)),
        "identf": np.eye(128, dtype=np.float32),
    }


def _make_in_maps(embedding, ln_w, ln_b, W1, b1, W2, b2):
    embedding = np.asarray(embedding, dtype=np.float32)
    weights = _prep_weights(
        np.asarray(ln_w, dtype=np.float32), np.asarray(ln_b, dtype=np.float32),
        np.asarray(W1, dtype=np.float32), np.asarray(b1, dtype=np.float32),
        np.asarray(W2, dtype=np.float32), np.asarray(b2, dtype=np.float32),
    )
    xb = _bf16(embedding)                        # bf16 cast (rounding only)
    return [
        {"xt": np.ascontiguousarray(xb[c * RPC : (c + 1) * RPC].T), **weights}
        for c in range(N_CORES)
    ]


def kernel(embedding, ln_w, ln_b, W1, b1, W2, b2):
    from concourse.bass_utils import run_bass_kernel_spmd

    in_maps = _make_in_maps(embedding, ln_w, ln_b, W1, b1, W2, b2)
    nc = _get_nc()
    res = run_bass_kernel_spmd(nc, in_maps, core_ids=list(range(N_CORES)))
    out = np.concatenate([res.results[c]["y"] for c in range(N_CORES)], axis=0)
    return out.astype(np.float32)


# revision 19
# speedup vs baseline: 1.5713x; 1.0724x over previous
"""Trainium2 Bass kernel for nn_ClassificationHead: LayerNorm -> Linear(1024,256) -> GELU -> Linear(256,2).

Data-parallel over 8 NeuronCores: each core processes 8192 rows of the
65536-row batch; the tiny weights are replicated. The host supplies each
core's shard pre-transposed in bf16 (layout-only prep: [1024, 8192],
K-major as the tensor engine requires); all math runs on device.

Per-core pipeline, per 512-row block (4 tiles of 128 rows):
  1. One DMA loads the K-major block [128, 8, 512] bf16.
  2. Per tile, TensorE runs 8 accumulating matmuls against W1aug
     ([W1' | ones] -> PSUM cols 0:256 = x @ W1', col 256 = rowsum), plus a
     Gram matmul reusing the already-loaded stationary x-chunk
     (ldweights=False) into PSUM cols 257:385.
  3. DVE extracts -mu (from the rowsum col) and sum(x^2) (Gram diagonal via
     tensor_tensor_reduce against an identity); a batched Newton-rsqrt
     (bit-trick seed + 2 iterations) gives g = 1/sqrt(var+eps) and
     rhat = 1/g. A tiny [128,128] xbar-transpose DMA flips the per-row
     stats into rows.
  4. TensorE adds the rank-2 correction (-mu ox s1 + rhat ox c1), so after
     the GELU's per-partition scale g the PSUM holds exactly LN(x)@W1'+b1.
  5. ACT evaluates exact GELU with scale g -> bf16 h tile.
  6. TensorE transposes h (via identity), ACT evacuates PSUM->SBUF bf16,
     TensorE computes h @ W2; DVE adds b2 into a staging tile.
  7. One DMA writes the [8192, 2] fp32 result back.

Host-side weight folding (tiny, O(1MB)): W1' = ln_w[:,None]*W1,
s1 = colsum(W1'), c1 = ln_b@W1 + b1.
"""
import sys

sys.path.insert(0, "/opt/trn_rl_repo")
sys.path.insert(0, "/root/.axon_site")

import numpy as np
import ml_dtypes

N_CORES = 8
BATCH = 65536
D = 1024
H = 256
OUT = 2
RPC = BATCH // N_CORES  # rows per core
NT = RPC // 128         # 128-row tiles per core
KC = D // 128           # contraction chunks
G = 4                   # tiles per block (512 rows)
NB = NT // G            # blocks per core
EPS = 1e-5
MAGIC = 0x5F3759DF

_cache = {}


def _bf16(a):
    return np.asarray(a, dtype=ml_dtypes.bfloat16)


def _build(rpc=RPC):
    import concourse.bacc as bacc
    from concourse.tile_rust import add_dep_helper
    import concourse.mybir as mybir
    from concourse import tile

    f32 = mybir.dt.float32
    i32 = mybir.dt.int32
    bf16 = mybir.dt.bfloat16
    AF = mybir.ActivationFunctionType
    ALU = mybir.AluOpType

    nc = bacc.Bacc(None, target_bir_lowering=False, debug=False)

    xt_in = nc.dram_tensor("xt", [D, rpc], bf16, kind="ExternalInput")
    w1_in = nc.dram_tensor("w1aug", [128, KC, H + 1], bf16, kind="ExternalInput")
    sc_in = nc.dram_tensor("screp", [2 * G, G, H + 1], bf16, kind="ExternalInput")
    w2_in = nc.dram_tensor("w2b", [128, 2, OUT], bf16, kind="ExternalInput")
    b2_in = nc.dram_tensor("b2c", [OUT, 1], f32, kind="ExternalInput")
    idb_in = nc.dram_tensor("identb", [128, 128], bf16, kind="ExternalInput")
    idf_in = nc.dram_tensor("identf", [128, 128], f32, kind="ExternalInput")
    y_out = nc.dram_tensor("y", [OUT, rpc], f32, kind="ExternalOutput")

    xt_v = xt_in.rearrange("(c p) r -> p c r", p=128)   # [128, KC, RPC]

    with tile.TileContext(nc) as tc:
        with (
            tc.tile_pool(name="wpool", bufs=1) as wp,
            tc.tile_pool(name="xtp", bufs=3) as xtp,
            tc.tile_pool(name="statp", bufs=2) as statp,
            tc.tile_pool(name="scrp", bufs=2) as scrp,
            tc.tile_pool(name="hbp", bufs=3) as hbp,
            tc.tile_pool(name="htp", bufs=3) as htp,
            tc.tile_pool(name="outp", bufs=1) as outp,
            tc.tile_pool(name="pszp", bufs=5, space="PSUM") as pszp,
            tc.tile_pool(name="psgp", bufs=1, space="PSUM") as psgp,
            tc.tile_pool(name="pstp", bufs=1, space="PSUM") as pstp,
            tc.tile_pool(name="psop", bufs=1, space="PSUM") as psop,
        ):
            w1sb = wp.tile([128, KC, H + 1], bf16)
            nc.sync.dma_start(w1sb[:], w1_in[:])
            scsb = wp.tile([2 * G, G, H + 1], bf16)
            nc.sync.dma_start(scsb[:], sc_in[:])
            w2sb = wp.tile([128, 2, OUT], bf16)
            nc.sync.dma_start(w2sb[:], w2_in[:])
            b2sb = wp.tile([OUT, 1], f32)
            nc.sync.dma_start(b2sb[:], b2_in[:])
            idbsb = wp.tile([128, 128], bf16)
            nc.sync.dma_start(idbsb[:], idb_in[:])
            idfsb = wp.tile([128, 128], f32)
            nc.sync.dma_start(idfsb[:], idf_in[:])

            nt = rpc // 128
            outsb = outp.tile([OUT, rpc], f32)

            for u in range(nt // G):
                xtg = xtp.tile([128, KC, G * 128], bf16, tag="xtg")
                nc.sync.dma_start(xtg[:], xt_v[:, :, u * G * 128 : (u + 1) * G * 128])

                S = statp.tile([128, G, 2], f32, tag="S")
                pszs = []
                for q in range(G):
                    rs = q * 128
                    pszg = pszp.tile([128, H + 1], f32, tag="pszg")
                    pszs.append(pszg)
                    psg = psgp.tile([128, 128], f32, tag="psg")
                    for k in range(KC):
                        mm1 = nc.tensor.matmul(
                            pszg[:, 0 : H + 1], xtg[:, k, rs : rs + 128], w1sb[:, k, :],
                            start=(k == 0), stop=False,
                        )
                        mmg = nc.tensor.matmul(
                            psg[:],
                            xtg[:, k, rs : rs + 128], xtg[:, k, rs : rs + 128],
                            start=(k == 0), stop=(k == KC - 1),
                        )
                        mmg.ins.ldweights = False
                        add_dep_helper(mm1.ins, mmg.ins, False, "gram reuses stationary")
                    # -mu and sum(x^2) into the per-block stats tile
                    nc.vector.tensor_scalar_mul(S[:, q, 0:1], pszg[:, H : H + 1], -1.0 / D)
                    scr = scrp.tile([128, 128], f32, tag="scr")
                    nc.vector.scalar_tensor_tensor(
                        scr[:], idfsb[:], 1.0, psg[:],
                        ALU.mult, ALU.mult, accum_out=S[:, q, 1:2],
                    )

                # Batched stats: V = var+eps = SS/D - mu^2 + eps; Y = rsqrt(V).
                A1 = statp.tile([128, G], f32, tag="A1")
                nc.vector.tensor_scalar(A1[:], S[:, :, 1], 1.0 / D, EPS, ALU.mult, ALU.add)
                B = statp.tile([128, G], f32, tag="B")
                nc.vector.tensor_tensor(B[:], S[:, :, 0], S[:, :, 0], ALU.mult)
                V = statp.tile([128, G], f32, tag="V")
                nc.vector.tensor_tensor(V[:], A1[:], B[:], ALU.subtract)
                Y = statp.tile([128, G], f32, tag="Y")
                T = statp.tile([128, G], f32, tag="T")
                nc.vector.tensor_scalar(T[:].bitcast(i32), V[:].bitcast(i32), 1, None, ALU.logical_shift_right)
                nc.vector.tensor_scalar(Y[:].bitcast(i32), T[:].bitcast(i32), -1, MAGIC, ALU.mult, ALU.add)
                for _ in range(2):
                    nc.vector.tensor_tensor(T[:], V[:], Y[:], ALU.mult)
                    nc.vector.tensor_tensor(T[:], T[:], Y[:], ALU.mult)
                    nc.vector.tensor_scalar(T[:], T[:], -0.5, 1.5, ALU.mult, ALU.add)
                    nc.vector.tensor_tensor(Y[:], Y[:], T[:], ALU.mult)

                # BM cols 0:2G = interleaved (-mu, rhat) in bf16; xbar-flip to rows.
                BM = scrp.tile([128, 128], bf16, tag="BM")
                BMv = BM[:, 0 : 2 * G].rearrange("p (q s) -> p q s", s=2)
                nc.vector.tensor_copy(BMv[:, :, 0], S[:, :, 0])
                nc.vector.tensor_tensor(BMv[:, :, 1], V[:], Y[:], ALU.mult)
                BMT = scrp.tile([128, 128], bf16, tag="BMT")
                nc.sync.dma_start(BMT[:], BM[:], transpose=True)

                for q in range(G):
                    t = u * G + q
                    pszg = pszs[q]
                    nc.tensor.matmul(
                        pszg[:, 0 : H + 1], BMT[0 : 2 * G, :],
                        scsb[:, q, :], start=False, stop=True,
                    )
                    hb = hbp.tile([128, H], bf16, tag="hb")
                    nc.scalar.activation(
                        hb[:], pszg[:, 0:H], AF.Gelu, bias=0.0, scale=Y[:, q : q + 1]
                    )

                    pst = pstp.tile([128, H], bf16, tag="pst")
                    nc.tensor.transpose(pst[:, 0:128], hb[:, 0:128], idbsb[:])
                    nc.tensor.transpose(pst[:, 128:256], hb[:, 128:256], idbsb[:])
                    ht = htp.tile([128, 2, 128], bf16, tag="ht")
                    nc.scalar.copy(ht[:], pst[:])

                    pso = psop.tile([OUT, 128], f32, tag="pso")
                    nc.tensor.matmul(pso[:], w2sb[:, 0, :], ht[:, 0, :], start=True, stop=False)
                    nc.tensor.matmul(pso[:], w2sb[:, 1, :], ht[:, 1, :], start=False, stop=True)

                    nc.vector.tensor_scalar_add(outsb[:, t * 128 : (t + 1) * 128], pso[:], b2sb[:, 0:1])

            nc.sync.dma_start(y_out[:], outsb[:])

    nc.finalize()
    return nc


def _get_nc():
    if "nc" not in _cache:
        _cache["nc"] = _build()
    return _cache["nc"]


def _prep_weights(ln_w, ln_b, W1, b1, W2, b2):
    W1p = ln_w[:, None] * W1                      # [1024, 256]
    s1 = W1p.sum(axis=0)                          # [256]
    c1 = ln_b @ W1 + b1                           # [256]
    w1aug = np.concatenate([W1p, np.ones((D, 1), np.float32)], axis=1)  # ones col -> rowsum
    sc = np.zeros((2 * G, G, H + 1), np.float32)
    for q in range(G):
        sc[2 * q, q, 0:H] = s1
        sc[2 * q + 1, q, 0:H] = c1
    return {
        "w1aug": _bf16(w1aug.reshape(KC, 128, H + 1).transpose(1, 0, 2)),
        "screp": _bf16(sc),
        "w2b": _bf16(W2.reshape(2, 128, OUT).transpose(1, 0, 2)),
        "b2c": b2.reshape(OUT, 1).astype(np.float32).copy(),
        "identb": _bf16(np.eye(128, dtype=np.float32)),
        "identf": np.eye(128, dtype=np.float32),
    }


def _make_in_maps(embedding, ln_w, ln_b, W1, b1, W2, b2):
    embedding = np.asarray(embedding, dtype=np.float32)
    weights = _prep_weights(
        np.asarray(ln_w, dtype=np.float32), np.asarray(ln_b, dtype=np.float32),
        np.asarray(W1, dtype=np.float32), np.asarray(b1, dtype=np.float32),
        np.asarray(W2, dtype=np.float32), np.asarray(b2, dtype=np.float32),
    )
    xb = _bf16(embedding)                        # bf16 cast (rounding only)
    return [
        {"xt": np.ascontiguousarray(xb[c * RPC : (c + 1) * RPC].T), **weights}
        for c in range(N_CORES)
    ]


def kernel(embedding, ln_w, ln_b, W1, b1, W2, b2):
    from concourse.bass_utils import run_bass_kernel_spmd

    in_maps = _make_in_maps(embedding, ln_w, ln_b, W1, b1, W2, b2)
    nc = _get_nc()
    res = run_bass_kernel_spmd(nc, in_maps, core_ids=list(range(N_CORES)))
    out = np.concatenate(
        [np.ascontiguousarray(res.results[c]["y"].T) for c in range(N_CORES)], axis=0
    )
    return out.astype(np.float32)


# revision 20
# speedup vs baseline: 1.7684x; 1.1254x over previous
"""Trainium2 Bass kernel for nn_ClassificationHead: LayerNorm -> Linear(1024,256) -> GELU -> Linear(256,2).

Data-parallel over 8 NeuronCores: each core processes 8192 rows of the
65536-row batch; the tiny weights are replicated. The host supplies each
core's shard pre-transposed in bf16 (layout-only prep: [1024, 8192],
K-major as the tensor engine requires); all math runs on device.

Per-core pipeline, per 512-row block (4 tiles of 128 rows):
  1. One DMA loads the K-major block [128, 8, 512] bf16.
  2. Per tile, TensorE runs 8 accumulating matmuls against W1aug
     ([W1' | ones] -> PSUM cols 0:256 = x @ W1', col 256 = rowsum), plus a
     Gram matmul reusing the already-loaded stationary x-chunk
     (ldweights=False) into PSUM cols 257:385.
  3. DVE extracts -mu (from the rowsum col) and sum(x^2) (Gram diagonal via
     tensor_tensor_reduce against an identity); a batched Newton-rsqrt
     (bit-trick seed + 2 iterations) gives g = 1/sqrt(var+eps) and
     rhat = 1/g. A tiny [128,128] xbar-transpose DMA flips the per-row
     stats into rows.
  4. TensorE adds the rank-2 correction (-mu ox s1 + rhat ox c1), so after
     the GELU's per-partition scale g the PSUM holds exactly LN(x)@W1'+b1.
  5. ACT evaluates exact GELU with scale g -> bf16 h tile.
  6. TensorE transposes h (via identity), ACT evacuates PSUM->SBUF bf16,
     TensorE computes h @ W2; DVE adds b2 into a staging tile.
  7. One DMA writes the [8192, 2] fp32 result back.

Host-side weight folding (tiny, O(1MB)): W1' = ln_w[:,None]*W1,
s1 = colsum(W1'), c1 = ln_b@W1 + b1.
"""
import sys

sys.path.insert(0, "/opt/trn_rl_repo")
sys.path.insert(0, "/root/.axon_site")

import numpy as np
import ml_dtypes

N_CORES = 8
BATCH = 65536
D = 1024
H = 256
OUT = 2
RPC = BATCH // N_CORES  # rows per core
NT = RPC // 128         # 128-row tiles per core
KC = D // 128           # contraction chunks
G = 4                   # tiles per block (512 rows)
NB = NT // G            # blocks per core
EPS = 1e-5
MAGIC = 0x5F3759DF

_cache = {}


def _bf16(a):
    return np.asarray(a, dtype=ml_dtypes.bfloat16)


def _build(rpc=RPC):
    import concourse.bacc as bacc
    from concourse.tile_rust import add_dep_helper
    import concourse.mybir as mybir
    from concourse import tile

    f32 = mybir.dt.float32
    i32 = mybir.dt.int32
    bf16 = mybir.dt.bfloat16
    AF = mybir.ActivationFunctionType
    ALU = mybir.AluOpType

    nc = bacc.Bacc(None, target_bir_lowering=False, debug=False)

    xt_in = nc.dram_tensor("xt", [D, rpc], bf16, kind="ExternalInput")
    w1_in = nc.dram_tensor("w1aug", [128, KC, H + 1], bf16, kind="ExternalInput")
    sc_in = nc.dram_tensor("screp", [2 * G, G, H + 1], bf16, kind="ExternalInput")
    w2_in = nc.dram_tensor("w2rep", [128, OUT, H], bf16, kind="ExternalInput")
    b2_in = nc.dram_tensor("b2g", [128, G * OUT], f32, kind="ExternalInput")
    idf_in = nc.dram_tensor("identf", [128, 128], f32, kind="ExternalInput")
    y_out = nc.dram_tensor("y", [rpc, OUT], f32, kind="ExternalOutput")

    xt_v = xt_in.rearrange("(c p) r -> p c r", p=128)   # [128, KC, RPC]

    with tile.TileContext(nc) as tc:
        with (
            tc.tile_pool(name="wpool", bufs=1) as wp,
            tc.tile_pool(name="xtp", bufs=4) as xtp,
            tc.tile_pool(name="statp", bufs=2) as statp,
            tc.tile_pool(name="scrp", bufs=2) as scrp,
            tc.tile_pool(name="hbp", bufs=3) as hbp,
            tc.tile_pool(name="htp", bufs=3) as htp,
            tc.tile_pool(name="outp", bufs=1) as outp,
            tc.tile_pool(name="pszp", bufs=6, space="PSUM") as pszp,
            tc.tile_pool(name="psgp", bufs=2, space="PSUM") as psgp,
        ):
            w1sb = wp.tile([128, KC, H + 1], bf16)
            nc.sync.dma_start(w1sb[:], w1_in[:])
            scsb = wp.tile([2 * G, G, H + 1], bf16)
            nc.sync.dma_start(scsb[:], sc_in[:])
            w2sb = wp.tile([128, OUT, H], bf16)
            nc.sync.dma_start(w2sb[:], w2_in[:])
            b2sb = wp.tile([128, G * OUT], f32)
            nc.sync.dma_start(b2sb[:], b2_in[:])
            idfsb = wp.tile([128, 128], f32)
            nc.sync.dma_start(idfsb[:], idf_in[:])

            nt = rpc // 128
            outsb = outp.tile([128, nt, OUT], f32)

            for u in range(nt // G):
                xtg = xtp.tile([128, KC, G * 128], bf16, tag="xtg")
                nc.sync.dma_start(xtg[:], xt_v[:, :, u * G * 128 : (u + 1) * G * 128])

                S = statp.tile([128, G, 2], f32, tag="S")
                OB = statp.tile([128, G, OUT], f32, tag="OB")
                pszs = []
                for q in range(G):
                    rs = q * 128
                    pszg = pszp.tile([128, H + 1], f32, tag="pszg")
                    pszs.append(pszg)
                    psg = psgp.tile([128, 128], f32, tag="psg")
                    for k in range(KC):
                        mm1 = nc.tensor.matmul(
                            pszg[:, 0 : H + 1], xtg[:, k, rs : rs + 128], w1sb[:, k, :],
                            start=(k == 0), stop=False,
                        )
                        mmg = nc.tensor.matmul(
                            psg[:],
                            xtg[:, k, rs : rs + 128], xtg[:, k, rs : rs + 128],
                            start=(k == 0), stop=(k == KC - 1),
                        )
                        mmg.ins.ldweights = False
                        add_dep_helper(mm1.ins, mmg.ins, False, "gram reuses stationary")
                    # -mu and sum(x^2) into the per-block stats tile
                    nc.vector.tensor_scalar_mul(S[:, q, 0:1], pszg[:, H : H + 1], -1.0 / D)
                    scr = scrp.tile([128, 128], f32, tag="scr")
                    nc.vector.scalar_tensor_tensor(
                        scr[:], idfsb[:], 1.0, psg[:],
                        ALU.mult, ALU.mult, accum_out=S[:, q, 1:2],
                    )

                # Batched stats: V = var+eps = SS/D - mu^2 + eps; Y = rsqrt(V).
                A1 = statp.tile([128, G], f32, tag="A1")
                nc.vector.tensor_scalar(A1[:], S[:, :, 1], 1.0 / D, EPS, ALU.mult, ALU.add)
                B = statp.tile([128, G], f32, tag="B")
                nc.vector.tensor_tensor(B[:], S[:, :, 0], S[:, :, 0], ALU.mult)
                V = statp.tile([128, G], f32, tag="V")
                nc.vector.tensor_tensor(V[:], A1[:], B[:], ALU.subtract)
                Y = statp.tile([128, G], f32, tag="Y")
                T = statp.tile([128, G], f32, tag="T")
                nc.vector.tensor_scalar(T[:].bitcast(i32), V[:].bitcast(i32), 1, None, ALU.logical_shift_right)
                nc.vector.tensor_scalar(Y[:].bitcast(i32), T[:].bitcast(i32), -1, MAGIC, ALU.mult, ALU.add)
                for _ in range(2):
                    nc.vector.tensor_tensor(T[:], V[:], Y[:], ALU.mult)
                    nc.vector.tensor_tensor(T[:], T[:], Y[:], ALU.mult)
                    nc.vector.tensor_scalar(T[:], T[:], -0.5, 1.5, ALU.mult, ALU.add)
                    nc.vector.tensor_tensor(Y[:], Y[:], T[:], ALU.mult)

                # BM cols 0:2G = interleaved (-mu, rhat) in bf16; xbar-flip to rows.
                BM = scrp.tile([128, 128], bf16, tag="BM")
                BMv = BM[:, 0 : 2 * G].rearrange("p (q s) -> p q s", s=2)
                nc.vector.tensor_copy(BMv[:, :, 0], S[:, :, 0])
                nc.vector.tensor_tensor(BMv[:, :, 1], V[:], Y[:], ALU.mult)
                BMT = scrp.tile([128, 128], bf16, tag="BMT")
                nc.sync.dma_start(BMT[:], BM[:], transpose=True)

                for q in range(G):
                    t = u * G + q
                    pszg = pszs[q]
                    nc.tensor.matmul(
                        pszg[:, 0 : H + 1], BMT[0 : 2 * G, :],
                        scsb[:, q, :], start=False, stop=True,
                    )
                    hb = hbp.tile([128, H], bf16, tag="hb")
                    nc.scalar.activation(
                        hb[:], pszg[:, 0:H], AF.Gelu, bias=0.0, scale=Y[:, q : q + 1]
                    )

                    scr2 = scrp.tile([128, H], f32, tag="scr2")
                    for c in range(OUT):
                        nc.vector.scalar_tensor_tensor(
                            scr2[:], hb[:], 1.0, w2sb[:, c, :],
                            ALU.mult, ALU.mult, accum_out=OB[:, q, c : c + 1],
                        )

                nc.vector.tensor_add(
                    outsb[:, u * G : (u + 1) * G, :].opt(),
                    OB[:].opt(), b2sb[:].rearrange("p (q c) -> p q c", c=OUT),
                )

            nc.sync.dma_start(y_out.rearrange("(t p) c -> p t c", p=128), outsb[:])


    nc.finalize()
    return nc


def _get_nc():
    if "nc" not in _cache:
        _cache["nc"] = _build()
    return _cache["nc"]


def _prep_weights(ln_w, ln_b, W1, b1, W2, b2):
    W1p = ln_w[:, None] * W1                      # [1024, 256]
    s1 = W1p.sum(axis=0)                          # [256]
    c1 = ln_b @ W1 + b1                           # [256]
    w1aug = np.concatenate([W1p, np.ones((D, 1), np.float32)], axis=1)  # ones col -> rowsum
    sc = np.zeros((2 * G, G, H + 1), np.float32)
    for q in range(G):
        sc[2 * q, q, 0:H] = s1
        sc[2 * q + 1, q, 0:H] = c1
    return {
        "w1aug": _bf16(w1aug.reshape(KC, 128, H + 1).transpose(1, 0, 2)),
        "screp": _bf16(sc),
        "w2rep": _bf16(np.broadcast_to(W2.T, (128, OUT, H))),
        "b2g": np.broadcast_to(np.tile(b2, G), (128, G * OUT)).astype(np.float32).copy(),
        "identf": np.eye(128, dtype=np.float32),
    }


def _make_in_maps(embedding, ln_w, ln_b, W1, b1, W2, b2):
    embedding = np.asarray(embedding, dtype=np.float32)
    weights = _prep_weights(
        np.asarray(ln_w, dtype=np.float32), np.asarray(ln_b, dtype=np.float32),
        np.asarray(W1, dtype=np.float32), np.asarray(b1, dtype=np.float32),
        np.asarray(W2, dtype=np.float32), np.asarray(b2, dtype=np.float32),
    )
    xb = _bf16(embedding)                        # bf16 cast (rounding only)
    return [
        {"xt": np.ascontiguousarray(xb[c * RPC : (c + 1) * RPC].T), **weights}
        for c in range(N_CORES)
    ]


def kernel(embedding, ln_w, ln_b, W1, b1, W2, b2):
    from concourse.bass_utils import run_bass_kernel_spmd

    in_maps = _make_in_maps(embedding, ln_w, ln_b, W1, b1, W2, b2)
    nc = _get_nc()
    res = run_bass_kernel_spmd(nc, in_maps, core_ids=list(range(N_CORES)))
    out = np.concatenate([res.results[c]["y"] for c in range(N_CORES)], axis=0)
    return out.astype(np.float32)


# revision 21
# speedup vs baseline: 1.8495x; 1.0458x over previous
"""Trainium2 Bass kernel for nn_ClassificationHead: LayerNorm -> Linear(1024,256) -> GELU -> Linear(256,2).

Data-parallel over 8 NeuronCores: each core processes 8192 rows of the
65536-row batch; the tiny weights are replicated. The host supplies each
core's shard pre-transposed in bf16 (layout-only prep: [1024, 8192],
K-major as the tensor engine requires); all math runs on device.

Per-core pipeline, per 512-row block (4 tiles of 128 rows):
  1. One DMA loads the K-major block [128, 8, 512] bf16.
  2. Per tile, TensorE runs 8 accumulating matmuls against W1aug
     ([W1' | ones] -> PSUM cols 0:256 = x @ W1', col 256 = rowsum), plus a
     Gram matmul reusing the already-loaded stationary x-chunk
     (ldweights=False) into PSUM cols 257:385.
  3. DVE extracts -mu (from the rowsum col) and sum(x^2) (Gram diagonal via
     tensor_tensor_reduce against an identity); a batched Newton-rsqrt
     (bit-trick seed + 2 iterations) gives g = 1/sqrt(var+eps) and
     rhat = 1/g. A tiny [128,128] xbar-transpose DMA flips the per-row
     stats into rows.
  4. TensorE adds the rank-2 correction (-mu ox s1 + rhat ox c1), so after
     the GELU's per-partition scale g the PSUM holds exactly LN(x)@W1'+b1.
  5. ACT evaluates exact GELU with scale g -> bf16 h tile.
  6. TensorE transposes h (via identity), ACT evacuates PSUM->SBUF bf16,
     TensorE computes h @ W2; DVE adds b2 into a staging tile.
  7. One DMA writes the [8192, 2] fp32 result back.

Host-side weight folding (tiny, O(1MB)): W1' = ln_w[:,None]*W1,
s1 = colsum(W1'), c1 = ln_b@W1 + b1.
"""
import sys

sys.path.insert(0, "/opt/trn_rl_repo")
sys.path.insert(0, "/root/.axon_site")

import numpy as np
import ml_dtypes

N_CORES = 8
BATCH = 65536
D = 1024
H = 256
OUT = 2
RPC = BATCH // N_CORES  # rows per core
NT = RPC // 128         # 128-row tiles per core
KC = D // 128           # contraction chunks
G = 4                   # tiles per block (512 rows)
NB = NT // G            # blocks per core
EPS = 1e-5
MAGIC = 0x5F3759DF

_cache = {}


def _bf16(a):
    return np.asarray(a, dtype=ml_dtypes.bfloat16)


def _build(rpc=RPC):
    import concourse.bacc as bacc
    from concourse.tile_rust import add_dep_helper
    import concourse.mybir as mybir
    from concourse import tile

    f32 = mybir.dt.float32
    i32 = mybir.dt.int32
    bf16 = mybir.dt.bfloat16
    AF = mybir.ActivationFunctionType
    ALU = mybir.AluOpType

    nc = bacc.Bacc(None, target_bir_lowering=False, debug=False)

    xt_in = nc.dram_tensor("xt", [D, rpc], bf16, kind="ExternalInput")
    w1_in = nc.dram_tensor("w1aug", [128, KC, H + 1], bf16, kind="ExternalInput")
    sc_in = nc.dram_tensor("screp", [2 * G, G, H + 1], bf16, kind="ExternalInput")
    w2_in = nc.dram_tensor("w2rep", [128, OUT, H], bf16, kind="ExternalInput")
    b2_in = nc.dram_tensor("b2g", [128, G * OUT], f32, kind="ExternalInput")
    idf_in = nc.dram_tensor("identf", [128, 128], f32, kind="ExternalInput")
    y_out = nc.dram_tensor("y", [rpc, OUT], f32, kind="ExternalOutput")

    xt_v = xt_in.rearrange("(c p) r -> p c r", p=128)   # [128, KC, RPC]

    with tile.TileContext(nc) as tc:
        with (
            tc.tile_pool(name="wpool", bufs=1) as wp,
            tc.tile_pool(name="xtp", bufs=4) as xtp,
            tc.tile_pool(name="statp", bufs=2) as statp,
            tc.tile_pool(name="scrp", bufs=2) as scrp,
            tc.tile_pool(name="hbp", bufs=3) as hbp,
            tc.tile_pool(name="htp", bufs=3) as htp,
            tc.tile_pool(name="outp", bufs=1) as outp,
            tc.tile_pool(name="pszp", bufs=7, space="PSUM") as pszp,
            tc.tile_pool(name="psgp", bufs=1, space="PSUM") as psgp,
        ):
            w1sb = wp.tile([128, KC, H + 1], bf16)
            nc.sync.dma_start(w1sb[:], w1_in[:])
            scsb = wp.tile([2 * G, G, H + 1], bf16)
            nc.sync.dma_start(scsb[:], sc_in[:])
            w2sb = wp.tile([128, OUT, H], bf16)
            nc.sync.dma_start(w2sb[:], w2_in[:])
            b2sb = wp.tile([128, G * OUT], f32)
            nc.sync.dma_start(b2sb[:], b2_in[:])
            idfsb = wp.tile([128, 128], f32)
            nc.sync.dma_start(idfsb[:], idf_in[:])

            nt = rpc // 128
            outsb = outp.tile([128, nt, OUT], f32)

            for u in range(nt // G):
                xtg = xtp.tile([128, KC, G * 128], bf16, tag="xtg")
                nc.sync.dma_start(xtg[:], xt_v[:, :, u * G * 128 : (u + 1) * G * 128])

                S = statp.tile([128, G, 2], f32, tag="S")
                OB = statp.tile([128, G, OUT], f32, tag="OB")
                pszs = []
                for q in range(G):
                    rs = q * 128
                    pszg = pszp.tile([128, H + 1], f32, tag="pszg")
                    pszs.append(pszg)
                    psg = psgp.tile([128, 128], f32, tag="psg")
                    for k in range(KC):
                        mm1 = nc.tensor.matmul(
                            pszg[:, 0 : H + 1], xtg[:, k, rs : rs + 128], w1sb[:, k, :],
                            start=(k == 0), stop=False,
                        )
                        mmg = nc.tensor.matmul(
                            psg[:],
                            xtg[:, k, rs : rs + 128], xtg[:, k, rs : rs + 128],
                            start=(k == 0), stop=(k == KC - 1),
                        )
                        mmg.ins.ldweights = False
                        add_dep_helper(mm1.ins, mmg.ins, False, "gram reuses stationary")
                    # -mu and sum(x^2) into the per-block stats tile
                    nc.vector.tensor_scalar_mul(S[:, q, 0:1], pszg[:, H : H + 1], -1.0 / D)
                    scr = scrp.tile([128, 128], f32, tag="scr")
                    nc.vector.scalar_tensor_tensor(
                        scr[:], idfsb[:], 1.0, psg[:],
                        ALU.mult, ALU.mult, accum_out=S[:, q, 1:2],
                    )

                # Batched stats: V = var+eps = SS/D - mu^2 + eps; Y = rsqrt(V).
                A1 = statp.tile([128, G], f32, tag="A1")
                nc.vector.tensor_scalar(A1[:], S[:, :, 1], 1.0 / D, EPS, ALU.mult, ALU.add)
                B = statp.tile([128, G], f32, tag="B")
                nc.vector.tensor_tensor(B[:], S[:, :, 0], S[:, :, 0], ALU.mult)
                V = statp.tile([128, G], f32, tag="V")
                nc.vector.tensor_tensor(V[:], A1[:], B[:], ALU.subtract)
                Y = statp.tile([128, G], f32, tag="Y")
                T = statp.tile([128, G], f32, tag="T")
                nc.vector.tensor_scalar(T[:].bitcast(i32), V[:].bitcast(i32), 1, None, ALU.logical_shift_right)
                nc.vector.tensor_scalar(Y[:].bitcast(i32), T[:].bitcast(i32), -1, MAGIC, ALU.mult, ALU.add)
                for _ in range(2):
                    nc.vector.tensor_tensor(T[:], V[:], Y[:], ALU.mult)
                    nc.vector.tensor_tensor(T[:], T[:], Y[:], ALU.mult)
                    nc.vector.tensor_scalar(T[:], T[:], -0.5, 1.5, ALU.mult, ALU.add)
                    nc.vector.tensor_tensor(Y[:], Y[:], T[:], ALU.mult)

                # BM cols 0:2G = interleaved (-mu, rhat) in bf16; xbar-flip to rows.
                BM = scrp.tile([128, 128], bf16, tag="BM")
                BMv = BM[:, 0 : 2 * G].rearrange("p (q s) -> p q s", s=2)
                nc.vector.tensor_copy(BMv[:, :, 0], S[:, :, 0])
                nc.vector.tensor_tensor(BMv[:, :, 1], V[:], Y[:], ALU.mult)
                BMT = scrp.tile([128, 128], bf16, tag="BMT")
                nc.sync.dma_start(BMT[:], BM[:], transpose=True)

                for q in range(G):
                    t = u * G + q
                    pszg = pszs[q]
                    nc.tensor.matmul(
                        pszg[:, 0 : H + 1], BMT[0 : 2 * G, :],
                        scsb[:, q, :], start=False, stop=True,
                    )
                    hb = hbp.tile([128, H], bf16, tag="hb")
                    nc.scalar.activation(
                        hb[:], pszg[:, 0:H], AF.Gelu, bias=0.0, scale=Y[:, q : q + 1]
                    )

                    scr2 = scrp.tile([128, H], f32, tag="scr2")
                    for c in range(OUT):
                        nc.vector.scalar_tensor_tensor(
                            scr2[:], hb[:], 1.0, w2sb[:, c, :],
                            ALU.mult, ALU.mult, accum_out=OB[:, q, c : c + 1],
                        )

                nc.vector.tensor_add(
                    outsb[:, u * G : (u + 1) * G, :].opt(),
                    OB[:].opt(), b2sb[:].rearrange("p (q c) -> p q c", c=OUT),
                )

            nc.sync.dma_start(y_out.rearrange("(t p) c -> p t c", p=128), outsb[:])


    nc.finalize()
    return nc


def _get_nc():
    if "nc" not in _cache:
        _cache["nc"] = _build()
    return _cache["nc"]


def _prep_weights(ln_w, ln_b, W1, b1, W2, b2):
    W1p = ln_w[:, None] * W1                      # [1024, 256]
    s1 = W1p.sum(axis=0)                          # [256]
    c1 = ln_b @ W1 + b1                           # [256]
    w1aug = np.concatenate([W1p, np.ones((D, 1), np.float32)], axis=1)  # ones col -> rowsum
    sc = np.zeros((2 * G, G, H + 1), np.float32)
    for q in range(G):
        sc[2 * q, q, 0:H] = s1
        sc[2 * q + 1, q, 0:H] = c1
    return {
        "w1aug": _bf16(w1aug.reshape(KC, 128, H + 1).transpose(1, 0, 2)),
        "screp": _bf16(sc),
        "w2rep": _bf16(np.broadcast_to(W2.T, (128, OUT, H))),
        "b2g": np.broadcast_to(np.tile(b2, G), (128, G * OUT)).astype(np.float32).copy(),
        "identf": np.eye(128, dtype=np.float32),
    }


def _make_in_maps(embedding, ln_w, ln_b, W1, b1, W2, b2):
    embedding = np.asarray(embedding, dtype=np.float32)
    weights = _prep_weights(
        np.asarray(ln_w, dtype=np.float32), np.asarray(ln_b, dtype=np.float32),
        np.asarray(W1, dtype=np.float32), np.asarray(b1, dtype=np.float32),
        np.asarray(W2, dtype=np.float32), np.asarray(b2, dtype=np.float32),
    )
    xb = _bf16(embedding)                        # bf16 cast (rounding only)
    return [
        {"xt": np.ascontiguousarray(xb[c * RPC : (c + 1) * RPC].T), **weights}
        for c in range(N_CORES)
    ]


def kernel(embedding, ln_w, ln_b, W1, b1, W2, b2):
    from concourse.bass_utils import run_bass_kernel_spmd

    in_maps = _make_in_maps(embedding, ln_w, ln_b, W1, b1, W2, b2)
    nc = _get_nc()
    res = run_bass_kernel_spmd(nc, in_maps, core_ids=list(range(N_CORES)))
    out = np.concatenate([res.results[c]["y"] for c in range(N_CORES)], axis=0)
    return out.astype(np.float32)
